# revision 19
# baseline (speedup 1.0000x reference)
"""CrossCCC loss kernel for Trainium2 (8 NeuronCores, sequence-parallel).

Math
----
reference computes, for lags n = 0..249:
    pred_n = [n zeros] ++ prediction[:T-n]
    ccc_n  = 2*cov(pred_n, gt) / (var_gt + var_pred_n + (mean_gt - mean_pred_n)^2)
    out    = 1 - mean_n(ccc_n)

Every lag statistic decomposes into lag-independent global sums plus tiny
suffix corrections (computed on host in float64); the only heavy term is
the raw cross-correlation X_n = sum_j p[j]*gt[j+n].  With j = 128*b + k:
    X_n = sum_k G[k, k+n],   G[k, s] = sum_b p[128b + k] * gt[128b + s]
for s in [0, 384): a Gram-style matmul contracting over the block axis.

Sharding: blocks split across 8 cores; each core holds p as a flat
[128, 1024] fp8 tile (row q = elements [1024q, 1024q+1024)) and gt as a
flat [128, 1280] tile (row q = elements [1024q, 1024q+1280), 256 halo).
The DoubleRow fp8 matmul takes 3D [128, 2, n] operands with K-pair
stride 512 -- overlapping windows of the flat tiles, built as explicit
access patterns.  The Gram is split by s-columns across two PSUM banks
(A: s<192, B: s>=192) so the PSUM->SBUF casts run concurrently on DVE
and ACT and the two output halves ship on separate DMA queues.

Raw bass (no Tile framework): explicit semaphores, one block per
engine.  g loads as main [0:1024) on the sync HWDGE queue + halo tail
[1024:1280) on the gpsimd SWDGE queue; p on the scalar HWDGE queue; the
matmuls that read the halo (B2, A3, B3) are ordered last.  Host does
all scalar statistics, suffix corrections, diagonal traces, and the
final formula in float64.
"""

import numpy as np

T = 1_000_000
N_CORES = 8
ROWS = 128           # SBUF partitions; also the k-lane count
COLS = 1024          # per-row elements; 4 DoubleRow column-tiles of 256
SHARD = ROWS * COLS  # 131072 elements of p per core
HALO = 256           # gt halo: max lag reach 249 rounded up
GCOLS = COLS + HALO  # 1280
NS = 384             # G free size: covers s = k + n, n<250, k<128
HNS = NS // 2        # 192: Gram column split between the two PSUM banks
NLAGS = 250

_compiled = None


def _build():
    import concourse.bass as bass
    import concourse.mybir as mybir

    fp8 = mybir.dt.float8e4
    f32 = mybir.dt.float32
    AP = bass.AP
    nc = bass.Bass("TRN2", target_bir_lowering=False)

    p_dram = nc.dram_tensor("p", [ROWS, 2, 512], fp8, kind="ExternalInput")
    g_dram = nc.dram_tensor("g", [ROWS, GCOLS], fp8, kind="ExternalInput")
    outg_dram = nc.dram_tensor("outg", [ROWS, NS], fp8, kind="ExternalOutput")

    with (
        nc.semaphore("s_p1") as s_p1,
        nc.semaphore("s_p2") as s_p2,
        nc.semaphore("s_g1") as s_g1,
        nc.semaphore("s_g2") as s_g2,
        nc.semaphore("s_mma") as s_mma,
        nc.semaphore("s_mmb") as s_mmb,
        nc.semaphore("s_ca") as s_ca,
        nc.semaphore("s_cb") as s_cb,
        nc.semaphore("s_oa") as s_oa,
        nc.semaphore("s_ob") as s_ob,
        # pb[q, i, m] = p[1024q + 512i + m]; gb is flat:
        # gb[q, c] = g[1024q + c] for c in [0, 1280) (256-elem halo)
        nc.sbuf_tensor("pb", [ROWS, 2, 512], fp8) as pb,
        nc.sbuf_tensor("gb", [ROWS, GCOLS], fp8) as gb,
        nc.sbuf_tensor("outg_sb", [ROWS, NS], fp8) as outg,
        nc.sbuf_tensor("scratch", [ROWS, 1], f32) as scratch,
        # full-bank PSUM allocations keep A and B in distinct banks so
        # DVE can read bank A while PE still writes bank B
        nc.psum_tensor("gramA", [ROWS, 512], f32) as gramA,
        nc.psum_tensor("gramB", [ROWS, 512], f32) as gramB,
    ):
        def pbv(t):  # lhsT: [128, 2, 128] DoubleRow slice
            return AP(pb, 128 * t, [[1024, ROWS], [512, 2], [1, 128]])

        def gbv(off):  # rhs: [128, 2, HNS] DoubleRow window of flat gb
            return AP(gb, off, [[GCOLS, ROWS], [512, 2], [1, HNS]])

        with nc.Block() as block:

            @block.sync
            def _(sync):
                # g main: flat cols [0, 1024) -- contiguous 1024B rows
                sync.dma_start(
                    AP(gb, 0, [[GCOLS, ROWS], [1, COLS]]),
                    AP(g_dram, 0, [[GCOLS, ROWS], [1, COLS]]),
                ).then_inc(s_g1, 16)
                # output store: completion is not waited on in-program;
                # the runtime's end-of-execution DMA-ring drain guarantees
                # the write lands before results are handed back
                sync.wait_ge(s_ca, 1)
                sync.wait_ge(s_cb, 1)
                sync.dma_start(
                    AP(outg_dram, 0, [[NS, ROWS], [1, NS]]),
                    AP(outg, 0, [[NS, ROWS], [1, NS]]),
                ).then_inc(s_oa, 16)

            @block.scalar
            def _(scalar):
                scalar.dma_start(
                    AP(pb, 0, [[1024, ROWS], [512, 2], [1, 512]]),
                    AP(p_dram, 0, [[1024, ROWS], [512, 2], [1, 512]]),
                ).then_inc(s_p1, 16)
                # g halo tail: flat cols [1024, 1280) -- A3/B2/B3 only
                scalar.dma_start(
                    AP(gb, COLS, [[GCOLS, ROWS], [1, HALO]]),
                    AP(g_dram, COLS, [[GCOLS, ROWS], [1, HALO]]),
                ).then_inc(s_g2, 16)
                # dummy 1-elem activation: hoists the ACT_TABLE_LOAD into
                # the input-DMA wait window instead of the output path
                scalar.activation(
                    AP(scratch, 0, [[1, ROWS], [1, 1]]),
                    AP(scratch, 0, [[1, ROWS], [1, 1]]),
                    mybir.ActivationFunctionType.Copy,
                )
                scalar.wait_ge(s_mmb, 1)
                scalar.activation(
                    AP(outg, HNS, [[NS, ROWS], [1, HNS]]),
                    AP(gramB, 0, [[512, ROWS], [1, HNS]]),
                    mybir.ActivationFunctionType.Copy,
                ).then_inc(s_cb, 1)

            @block.tensor
            def _(tensor):
                DR = mybir.MatmulPerfMode.DoubleRow
                outA = AP(gramA, 0, [[512, ROWS], [1, HNS]])
                outB = AP(gramB, 0, [[512, ROWS], [1, HNS]])
                # A-chain first so cast A + output A overlap the B-chain.
                # Flat-g window for matmul X_t starts at col 128t (+HNS
                # for B); the i=1 sub-window [512+128t, 512+128t+HNS+192)
                # crosses col 1024 (the halo) for A3, B2, B3 only.
                tensor.wait_ge(s_p1, 16)
                tensor.wait_ge(s_g1, 16)
                tensor.matmul(outA, pbv(0), gbv(0),
                              start=True, stop=False, perf_mode=DR)
                tensor.matmul(outA, pbv(1), gbv(128),
                              start=False, stop=False, perf_mode=DR)
                tensor.matmul(outA, pbv(2), gbv(256),
                              start=False, stop=False, perf_mode=DR)
                tensor.wait_ge(s_g2, 16)
                tensor.matmul(outA, pbv(3), gbv(384),
                              start=False, stop=True, perf_mode=DR
                              ).then_inc(s_mma, 1)
                tensor.matmul(outB, pbv(0), gbv(HNS),
                              start=True, stop=False, perf_mode=DR)
                tensor.matmul(outB, pbv(1), gbv(128 + HNS),
                              start=False, stop=False, perf_mode=DR)
                tensor.matmul(outB, pbv(2), gbv(256 + HNS),
                              start=False, stop=False, perf_mode=DR)
                tensor.matmul(outB, pbv(3), gbv(384 + HNS),
                              start=False, stop=True, perf_mode=DR
                              ).then_inc(s_mmb, 1)

            @block.vector
            def _(vector):
                vector.wait_ge(s_mma, 1)
                vector.tensor_copy(
                    AP(outg, 0, [[NS, ROWS], [1, HNS]]),
                    AP(gramA, 0, [[512, ROWS], [1, HNS]]),
                ).then_inc(s_ca, 1)

    nc.finalize()
    return nc


def _get_compiled():
    global _compiled
    if _compiled is None:
        _compiled = _build()
    return _compiled


def _shard_inputs(p: np.ndarray, g: np.ndarray):
    import ml_dtypes

    f8 = ml_dtypes.float8_e4m3
    p_pad = np.zeros(N_CORES * SHARD, f8)
    p_pad[:T] = p.astype(f8)
    g_pad = np.zeros(N_CORES * SHARD + HALO, f8)
    g_pad[:T] = g.astype(f8)
    in_maps = []
    for c in range(N_CORES):
        p3 = p_pad[c * SHARD : (c + 1) * SHARD].reshape(ROWS, 2, 512)
        base = g_pad[c * SHARD : c * SHARD + SHARD + HALO]
        g2 = np.lib.stride_tricks.as_strided(
            base, shape=(ROWS, GCOLS), strides=(COLS, 1)
        )
        in_maps.append({"p": p3, "g": np.ascontiguousarray(g2)})
    return in_maps


def _finish(results, p: np.ndarray, g: np.ndarray):
    """Host-side float64 finish: sum the 8 partial Grams, take diagonal
    traces, add the exact global statistics and suffix corrections."""
    G = np.zeros((ROWS, NS), np.float64)
    for r in results:
        G += r["outg"].astype(np.float64)
    X = np.array([np.trace(G, offset=n) for n in range(NLAGS)])

    p64 = p.astype(np.float64)
    g64 = g.astype(np.float64)
    S_p = p64.sum()
    S_g = g64.sum()
    Q_p = np.dot(p64, p64)
    Q_g = np.dot(g64, g64)

    tail = p64[T - NLAGS + 1 :][::-1]  # last 249 elements, reversed
    R = np.concatenate([[0.0], np.cumsum(tail)])        # R[n], n=0..249
    R2 = np.concatenate([[0.0], np.cumsum(tail * tail)])

    m = S_g / T
    var_g = (Q_g - T * m * m) / (T - 1)
    Sv = S_g - T * m

    sum_n = S_p - R
    mp = sum_n / T
    sumsq_n = Q_p - R2
    var_p = (sumsq_n - T * mp * mp) / (T - 1)
    cov = (X - m * sum_n - mp * Sv) / T
    denom = var_g + var_p + (m - mp) ** 2
    ccc = 2.0 * cov / denom
    return np.float32(1.0 - ccc.mean())


def kernel(prediction: np.ndarray, ground_truth: np.ndarray) -> np.ndarray:
    from concourse import bass_utils

    p = np.asarray(prediction, np.float32).reshape(-1)
    g = np.asarray(ground_truth, np.float32).reshape(-1)
    assert p.shape == (T,) and g.shape == (T,)

    nc = _get_compiled()
    in_maps = _shard_inputs(p, g)
    res = bass_utils.run_bass_kernel_spmd(nc, in_maps, core_ids=list(range(N_CORES)))
    return _finish(res.results, p, g)


# revision 23
# speedup vs baseline: 1.0417x; 1.0417x over previous
"""CrossCCC loss kernel for Trainium2 (8 NeuronCores, sequence-parallel).

Math
----
reference computes, for lags n = 0..249:
    pred_n = [n zeros] ++ prediction[:T-n]
    ccc_n  = 2*cov(pred_n, gt) / (var_gt + var_pred_n + (mean_gt - mean_pred_n)^2)
    out    = 1 - mean_n(ccc_n)

Every lag statistic decomposes into lag-independent global sums plus tiny
suffix corrections (computed on host in float64); the only heavy term is
the raw cross-correlation X_n = sum_j p[j]*gt[j+n].  With j = 128*b + k:
    X_n = sum_k G[k, k+n],   G[k, s] = sum_b p[128b + k] * gt[128b + s]
for s in [0, 384): a Gram-style matmul contracting over the block axis.

Sharding: blocks split across 8 cores; each core holds p as a flat
[128, 1024] fp8 tile (row q = elements [1024q, 1024q+1024)) and gt as a
flat [128, 1280] tile (row q = elements [1024q, 1024q+1280), 256 halo).
The DoubleRow fp8 matmul takes 3D [128, 2, n] operands with K-pair
stride 512 -- overlapping windows of the flat tiles, built as explicit
access patterns.  The Gram is split by s-columns across two PSUM banks
(A: s<192, B: s>=192) so the PSUM->SBUF casts run concurrently on DVE
and ACT and the two output halves ship on separate DMA queues.

Raw bass (no Tile framework): explicit semaphores, one block per
engine.  g loads as main [0:1024) on the sync HWDGE queue + halo tail
[1024:1280) on the gpsimd SWDGE queue; p on the scalar HWDGE queue; the
matmuls that read the halo (B2, A3, B3) are ordered last.  Host does
all scalar statistics, suffix corrections, diagonal traces, and the
final formula in float64.
"""

import numpy as np

T = 1_000_000
N_CORES = 8
ROWS = 128           # SBUF partitions; also the k-lane count
COLS = 1024          # per-row elements; 4 DoubleRow column-tiles of 256
SHARD = ROWS * COLS  # 131072 elements of p per core
HALO = 256           # gt halo: max lag reach 249 rounded up
GCOLS = COLS + HALO  # 1280
NS = 384             # G free size: covers s = k + n, n<250, k<128
HNS = NS // 2        # 192: Gram column split between the two PSUM banks
NLAGS = 250

_compiled = None


def _build():
    import concourse.bass as bass
    import concourse.mybir as mybir

    fp8 = mybir.dt.float8e4
    f32 = mybir.dt.float32
    AP = bass.AP
    nc = bass.Bass("TRN2", target_bir_lowering=False)

    bf16 = mybir.dt.bfloat16
    p_dram = nc.dram_tensor("p", [ROWS, 2, 512], fp8, kind="ExternalInput")
    g_dram = nc.dram_tensor("g", [ROWS, GCOLS], fp8, kind="ExternalInput")
    outg_dram = nc.dram_tensor("outg", [ROWS, NS], bf16, kind="ExternalOutput")

    with (
        nc.semaphore("s_p1") as s_p1,
        nc.semaphore("s_p2") as s_p2,
        nc.semaphore("s_g1") as s_g1,
        nc.semaphore("s_g2") as s_g2,
        nc.semaphore("s_mma") as s_mma,
        nc.semaphore("s_mmb") as s_mmb,
        nc.semaphore("s_ca") as s_ca,
        nc.semaphore("s_cb") as s_cb,
        nc.semaphore("s_oa") as s_oa,
        nc.semaphore("s_ob") as s_ob,
        # pb[q, i, m] = p[1024q + 512i + m]; gb is flat:
        # gb[q, c] = g[1024q + c] for c in [0, 1280) (256-elem halo)
        nc.sbuf_tensor("pb", [ROWS, 2, 512], fp8) as pb,
        nc.sbuf_tensor("gb", [ROWS, GCOLS], fp8) as gb,
        nc.sbuf_tensor("outg_sb", [ROWS, NS], bf16) as outg,
        # full-bank PSUM allocations keep A and B in distinct banks so
        # DVE can read bank A while PE still writes bank B
        nc.psum_tensor("gramA", [ROWS, 512], f32) as gramA,
        nc.psum_tensor("gramB", [ROWS, 512], f32) as gramB,
    ):
        def pbv(t):  # lhsT: [128, 2, 128] DoubleRow slice
            return AP(pb, 128 * t, [[1024, ROWS], [512, 2], [1, 128]])

        def gbv(off):  # rhs: [128, 2, HNS] DoubleRow window of flat gb
            return AP(gb, off, [[GCOLS, ROWS], [512, 2], [1, HNS]])

        with nc.Block() as block:

            @block.sync
            def _(sync):
                # g main: flat cols [0, 1024) -- contiguous 1024B rows
                sync.dma_start(
                    AP(gb, 0, [[GCOLS, ROWS], [1, COLS]]),
                    AP(g_dram, 0, [[GCOLS, ROWS], [1, COLS]]),
                ).then_inc(s_g1, 16)
                # output store: completion is not waited on in-program;
                # the runtime's end-of-execution DMA-ring drain guarantees
                # the write lands before results are handed back
                sync.wait_ge(s_cb, 1)
                sync.dma_start(
                    AP(outg_dram, 0, [[NS, ROWS], [1, NS]]),
                    AP(outg, 0, [[NS, ROWS], [1, NS]]),
                ).then_inc(s_oa, 16)

            @block.scalar
            def _(scalar):
                scalar.dma_start(
                    AP(pb, 0, [[1024, ROWS], [512, 2], [1, 512]]),
                    AP(p_dram, 0, [[1024, ROWS], [512, 2], [1, 512]]),
                ).then_inc(s_p1, 16)
                # g halo tail: flat cols [1024, 1280) -- A3/B2/B3 only
                scalar.dma_start(
                    AP(gb, COLS, [[GCOLS, ROWS], [1, HALO]]),
                    AP(g_dram, COLS, [[GCOLS, ROWS], [1, HALO]]),
                ).then_inc(s_g2, 16)

            @block.tensor
            def _(tensor):
                DR = mybir.MatmulPerfMode.DoubleRow
                outA = AP(gramA, 0, [[512, ROWS], [1, HNS]])
                outB = AP(gramB, 0, [[512, ROWS], [1, HNS]])
                # A-chain first so cast A + output A overlap the B-chain.
                # Flat-g window for matmul X_t starts at col 128t (+HNS
                # for B); the i=1 sub-window [512+128t, 512+128t+HNS+192)
                # crosses col 1024 (the halo) for A3, B2, B3 only.
                tensor.wait_ge(s_p1, 16)
                tensor.wait_ge(s_g1, 16)
                tensor.matmul(outA, pbv(0), gbv(0),
                              start=True, stop=False, perf_mode=DR)
                tensor.matmul(outA, pbv(1), gbv(128),
                              start=False, stop=False, perf_mode=DR)
                tensor.matmul(outA, pbv(2), gbv(256),
                              start=False, stop=False, perf_mode=DR)
                tensor.wait_ge(s_g2, 16)
                tensor.matmul(outA, pbv(3), gbv(384),
                              start=False, stop=True, perf_mode=DR
                              ).then_inc(s_mma, 1)
                tensor.matmul(outB, pbv(0), gbv(HNS),
                              start=True, stop=False, perf_mode=DR)
                tensor.matmul(outB, pbv(1), gbv(128 + HNS),
                              start=False, stop=False, perf_mode=DR)
                tensor.matmul(outB, pbv(2), gbv(256 + HNS),
                              start=False, stop=False, perf_mode=DR)
                tensor.matmul(outB, pbv(3), gbv(384 + HNS),
                              start=False, stop=True, perf_mode=DR
                              ).then_inc(s_mmb, 1)

            @block.vector
            def _(vector):
                vector.wait_ge(s_mma, 1)
                vector.tensor_copy(
                    AP(outg, 0, [[NS, ROWS], [1, HNS]]),
                    AP(gramA, 0, [[512, ROWS], [1, HNS]]),
                ).then_inc(s_ca, 1)
                vector.wait_ge(s_mmb, 1)
                vector.tensor_copy(
                    AP(outg, HNS, [[NS, ROWS], [1, HNS]]),
                    AP(gramB, 0, [[512, ROWS], [1, HNS]]),
                ).then_inc(s_cb, 1)

    nc.finalize()
    return nc


def _get_compiled():
    global _compiled
    if _compiled is None:
        _compiled = _build()
    return _compiled


def _shard_inputs(p: np.ndarray, g: np.ndarray):
    import ml_dtypes

    f8 = ml_dtypes.float8_e4m3
    p_pad = np.zeros(N_CORES * SHARD, f8)
    p_pad[:T] = p.astype(f8)
    g_pad = np.zeros(N_CORES * SHARD + HALO, f8)
    g_pad[:T] = g.astype(f8)
    in_maps = []
    for c in range(N_CORES):
        p3 = p_pad[c * SHARD : (c + 1) * SHARD].reshape(ROWS, 2, 512)
        base = g_pad[c * SHARD : c * SHARD + SHARD + HALO]
        g2 = np.lib.stride_tricks.as_strided(
            base, shape=(ROWS, GCOLS), strides=(COLS, 1)
        )
        in_maps.append({"p": p3, "g": np.ascontiguousarray(g2)})
    return in_maps


def _finish(results, p: np.ndarray, g: np.ndarray):
    """Host-side float64 finish: sum the 8 partial Grams, take diagonal
    traces, add the exact global statistics and suffix corrections."""
    G = np.zeros((ROWS, NS), np.float64)
    for r in results:
        G += r["outg"].astype(np.float64)
    X = np.array([np.trace(G, offset=n) for n in range(NLAGS)])

    p64 = p.astype(np.float64)
    g64 = g.astype(np.float64)
    S_p = p64.sum()
    S_g = g64.sum()
    Q_p = np.dot(p64, p64)
    Q_g = np.dot(g64, g64)

    tail = p64[T - NLAGS + 1 :][::-1]  # last 249 elements, reversed
    R = np.concatenate([[0.0], np.cumsum(tail)])        # R[n], n=0..249
    R2 = np.concatenate([[0.0], np.cumsum(tail * tail)])

    m = S_g / T
    var_g = (Q_g - T * m * m) / (T - 1)
    Sv = S_g - T * m

    sum_n = S_p - R
    mp = sum_n / T
    sumsq_n = Q_p - R2
    var_p = (sumsq_n - T * mp * mp) / (T - 1)
    cov = (X - m * sum_n - mp * Sv) / T
    denom = var_g + var_p + (m - mp) ** 2
    ccc = 2.0 * cov / denom
    return np.float32(1.0 - ccc.mean())


def kernel(prediction: np.ndarray, ground_truth: np.ndarray) -> np.ndarray:
    from concourse import bass_utils

    p = np.asarray(prediction, np.float32).reshape(-1)
    g = np.asarray(ground_truth, np.float32).reshape(-1)
    assert p.shape == (T,) and g.shape == (T,)

    nc = _get_compiled()
    in_maps = _shard_inputs(p, g)
    res = bass_utils.run_bass_kernel_spmd(nc, in_maps, core_ids=list(range(N_CORES)))
    return _finish(res.results, p, g)
